# revision 19
# baseline (speedup 1.0000x reference)
"""AdaMemAttention Trainium2 kernel (8 NeuronCores, SPMD) — v3.

Sharding: core c -> (batch b = c//2, head-group hg = c%2, heads hg*6..+6).
Tokens host-permuted per core to [own-half | other-half] so the SPMD
program is hg-invariant.

Pipeline per core:
  A: x/w loads issued up-front (x on the Act DMA queue, K-bank loads on
     SP), x-transposes, q1
  S: memory scoring in two staged passes (bank rows, then prev rows);
     per-pair exact top-k selection (kth_largest threshold -> mask*iota
     -> sparse_gather) pipelined into the scoring loop
  C: qkv GEMM (fused k|v dma_gathers run under it on Pool/DMA)
  H: single-pass attention over [current 1568 | selected 511] keys,
     software-pipelined (2-group score lookahead keeps Activation busy);
     other-half chunks first, exchanged via masked zero-slot
     ReduceScatter hidden under the own-half chunks
  P: projection of own 784 tokens x full 12 heads + bias
"""
import sys
sys.path.insert(0, "/opt/trn_rl_repo")
import numpy as np

B, N, C, H, D = 4, 1568, 768, 12, 64
NB, NP = 2048, 1568
NM = NB + NP
KB, KP = 153, 358
HL = 6
SEL = KB + KP              # 511
NH = N // 2                # 784

_cache = {}


def _build():
    import concourse.bass as bass
    import concourse.bacc as bacc
    import concourse.mybir as mybir
    import concourse.tile as tile

    dt = mybir.dt
    Alu = mybir.AluOpType
    Act = mybir.ActivationFunctionType
    f32, f32r, i16, u32 = dt.float32, dt.float32r, dt.int16, dt.uint32

    nc = bacc.Bacc("TRN2", target_bir_lowering=False, debug=False, num_devices=8)

    x_d = nc.dram_tensor("x", [N, C], f32, kind="ExternalInput")
    x0_d = nc.dram_tensor("x0", [1, C], f32, kind="ExternalInput")
    wqkvT_d = nc.dram_tensor("wqkvT", [C, 1152], f32, kind="ExternalInput")
    wprojT_d = nc.dram_tensor("wprojT", [C, C], f32, kind="ExternalInput")
    bproj_d = nc.dram_tensor("bproj", [1, C], f32, kind="ExternalInput")
    memkv_d = nc.dram_tensor("memkv", [HL, NM, 2 * D], f32,
                             kind="ExternalInput")
    ident_d = nc.dram_tensor("ident", [128, 128], f32, kind="ExternalInput")
    iota_d = nc.dram_tensor("iota226", [16, 226], f32, kind="ExternalInput")
    msk_d = nc.dram_tensor("msk", [128, 2], f32, kind="ExternalInput")

    out_d = nc.dram_tensor("out", [NH, C], f32, kind="ExternalOutput")
    import os
    DBG = os.environ.get("KDBG", "0") == "1"
    if DBG:
        dbg_sct = nc.dram_tensor("dbg_sct", [66, NM], f32, kind="ExternalOutput")
        dbg_thr = nc.dram_tensor("dbg_thr", [1, 24], f32, kind="ExternalOutput")
        dbg_idx = nc.dram_tensor("dbg_idx", [16, HL, 32], f32,
                                 kind="ExternalOutput")
        dbg_aT = nc.dram_tensor("dbg_aT", [128, 3, N], f32, kind="ExternalOutput")

    AQT = [(NH, 512), (NH + 512, 272), (0, 512), (512, 272)]
    MC = [128] * 12 + [32] + [128, 128, 128, 127]
    NCH = 13
    QBANK = 1.0 - (KB - 0.5) / (NB - 1)
    QPREV = 1.0 - (KP - 0.5) / (NP - 1)

    with tile.TileContext(nc) as tc, \
         tc.tile_pool(name="cst", bufs=1) as cst, \
         tc.tile_pool(name="dram", bufs=1, space="DRAM") as dram, \
         tc.tile_pool(name="bigB", bufs=1) as bigB:

        ident = cst.tile([128, 128], f32); nc.sync.dma_start(ident[:], ident_d[:])
        iota226 = cst.tile([16, 226], f32)
        nc.scalar.dma_start(iota226[:], iota_d[:])
        msk = cst.tile([128, 2], f32); nc.scalar.dma_start(msk[:], msk_d[:])
        ones1f = cst.tile([1, 128], f32)
        nc.vector.memset(ones1f[:], 1.0)
        ones1 = cst.tile([1, 128], f32r)
        nc.vector.tensor_copy(ones1[:], ones1f[:])
        q1 = cst.tile([128, 3], f32)
        q1blk = cst.tile([128, 6], f32)

        qT = bigB.tile([128, 3, N], f32r)
        kTc = bigB.tile([128, 3, N], f32r)
        kTs = bigB.tile([128, 3, 512], f32r)
        v_cur = bigB.tile([128, 13, HL, 65], f32r)
        v_sel = bigB.tile([128, 4, HL, 65], f32r)

        a_send = dram.tile([2, 128, 3, NH], f32)
        a_recv = dram.tile([128, 3, NH], f32)

        with tc.tile_pool(name="scA", bufs=1) as scA:
            xTr = scA.tile([128, 6, N], f32r)
            wqr = scA.tile([128, 6, 1152], f32r)
            kvsel = scA.tile([128, HL, 4, 128], f32)

            with tc.tile_pool(name="scS", bufs=1) as scS:
                sct = scS.tile([66, NM], f32)
                kbt = scS.tile([128, 12, 16], f32)
                s16 = scS.tile([16, HL, 226], f32)
                thr12 = scS.tile([1, 24], f32)
                thrB = scS.tile([16, 24], f32)
                selall = scS.tile([16, HL, 32], f32)
                idxs = scS.tile([128, HL, 32], i16)
                nfound = scS.tile([1, 16], u32)

                # ===== phase A =====
                nc.vector.memset(kbt[:], -1.0e30)
                nc.vector.memset(selall[:], 0.0)
                with tc.tile_pool(name="pA", bufs=1) as pA, \
                     tc.tile_pool(name="psA", bufs=2, space="PSUM") as psA:
                    # weights first on SP (needed by q1); bank loads follow
                    wqc = pA.tile([128, 6, 1152], f32)
                    nc.sync.dma_start(
                        wqc[:],
                        wqkvT_d[:].rearrange("(cc p) f -> p cc f", p=128))
                    x0 = pA.tile([128, 6], f32)
                    nc.scalar.dma_start(
                        x0[:],
                        x0_d[0:1, :].rearrange("one (cc p) -> p (one cc)",
                                               p=128))
                    for cc in range(6):
                        nc.vector.tensor_copy(wqr[:, cc, :], wqc[:, cc, :])
                    for fq in range(3):
                        q1ps = psA.tile([128, 1], f32, space="PSUM",
                                        tag="q1ps", name="q1ps")
                        for cc in range(6):
                            nc.tensor.matmul(
                                q1ps[:], wqc[:, cc, 128 * fq:128 * (fq + 1)],
                                x0[:, cc:cc + 1],
                                start=(cc == 0), stop=(cc == 5))
                        nc.vector.tensor_copy(q1[:, fq:fq + 1], q1ps[:])
                    nc.vector.memset(q1blk[:], 0.0)
                    for h in range(HL):
                        hb = 64 * (h % 2)
                        nc.vector.tensor_copy(
                            q1blk[hb:hb + 64, h:h + 1],
                            q1[hb:hb + 64, h // 2:h // 2 + 1])
                    for c in range(NCH):
                        rows = 128 if c < 12 else 32
                        xc = pA.tile([128, C], f32, tag="xc", name="xc",
                                     bufs=3)
                        nc.scalar.dma_start(xc[0:rows, :],
                                            x_d[128 * c:128 * c + rows, :])
                        for cc in range(6):
                            tpx = psA.tile([128, 128], f32, space="PSUM",
                                           tag="tpx", name="tpx", bufs=2)
                            nc.tensor.transpose(
                                tpx[0:128, 0:rows],
                                xc[0:rows, 128 * cc:128 * (cc + 1)],
                                ident[0:rows, 0:rows])
                            nc.vector.tensor_copy(
                                xTr[:, cc, 128 * c:128 * c + rows],
                                tpx[0:128, 0:rows])

                # ===== phase S: staged scoring + per-pair selection =====
                with tc.tile_pool(name="pS", bufs=1) as pS, \
                     tc.tile_pool(name="psS", bufs=2, space="PSUM") as psS:
                    for stage in range(2):
                        r0 = 0 if stage == 0 else NB
                        nrow = NB if stage == 0 else NP
                        for j in range(3):
                            bp = pS.tile([128, 2, 16, 64], f32, tag="bp",
                                         name="bp", bufs=2)
                            for hh in range(2):
                                h = 2 * j + hh
                                nfull = nrow // 128
                                nc.sync.dma_start(
                                    bp[:, hh, 0:nfull, :],
                                    memkv_d[h, r0:r0 + 128 * nfull, 0:64]
                                    .rearrange("(c p) e -> p c e", p=128))
                                if nrow % 128:
                                    nc.sync.dma_start(
                                        bp[0:nrow % 128, hh, nfull, :],
                                        memkv_d[h, r0 + 128 * nfull:r0 + nrow,
                                                0:64])
                            for c5 in range((nrow + 511) // 512):
                                w = min(512, nrow - 512 * c5)
                                kT5 = pS.tile([128, 512], f32, tag="kT5",
                                              name="kT5", bufs=2)
                                ncb = (w + 127) // 128
                                for hh in range(2):
                                    for cb in range(ncb):
                                        rows = min(128, w - 128 * cb)
                                        tpk = psS.tile([64, 128], f32,
                                                       space="PSUM", tag="tpk",
                                                       name="tpk", bufs=3)
                                        nc.tensor.transpose(
                                            tpk[0:64, 0:rows],
                                            bp[0:rows, hh, 4 * c5 + cb, :],
                                            ident[0:rows, 0:rows])
                                        nc.vector.tensor_copy(
                                            kT5[64 * hh:64 * hh + 64,
                                                128 * cb:128 * cb + rows],
                                            tpk[0:64, 0:rows])
                                scps = psS.tile([2, 512], f32, space="PSUM",
                                                tag="scps", name="scps", bufs=2)
                                nc.tensor.matmul(
                                    scps[0:2, 0:w],
                                    q1blk[:, 2 * j:2 * j + 2],
                                    kT5[:, 0:w],
                                    start=True, stop=True)
                                nc.vector.tensor_copy(
                                    sct[32 * j:32 * j + 2,
                                        r0 + 512 * c5:r0 + 512 * c5 + w],
                                    scps[0:2, 0:w])
                            if stage == 0:
                                continue
                            # both score rows complete: selection for pair j
                            for hh in range(2):
                                h = 2 * j + hh
                                p = 32 * j + hh
                                nc.scalar.dma_start(
                                    kbt[:, h, 0:16],
                                    sct[p:p + 1, 0:NB].rearrange(
                                        "one (p2 f) -> one p2 f", p2=128))
                                nc.scalar.dma_start(
                                    kbt[:, 6 + h, 0:12],
                                    sct[p:p + 1, NB:NB + 1536].rearrange(
                                        "one (p2 f) -> one p2 f", p2=128))
                                nc.scalar.dma_start(
                                    kbt[0:32, 6 + h, 12:13],
                                    sct[p:p + 1, NB + 1536:NM].rearrange(
                                        "one (p2 f) -> one p2 f", p2=32))
                                nc.scalar.dma_start(
                                    s16[:, h, 0:128],
                                    sct[p:p + 1, 0:NB].rearrange(
                                        "one (p2 f) -> one p2 f", p2=16))
                                nc.scalar.dma_start(
                                    s16[:, h, 128:226],
                                    sct[p:p + 1, NB:NM].rearrange(
                                        "one (p2 f) -> one p2 f", p2=16))
                                nc.gpsimd.kth_largest(
                                    thr12[0:1, 2 * h:2 * h + 2],
                                    kbt[:, h, 0:16], 16, KB, quantile=QBANK)
                                nc.gpsimd.kth_largest(
                                    thr12[0:1, 12 + 2 * h:14 + 2 * h],
                                    kbt[:, 6 + h, 0:13], 13, KP,
                                    quantile=QPREV)
                                nc.gpsimd.partition_broadcast(
                                    thrB[:, 2 * h:2 * h + 2],
                                    thr12[0:1, 2 * h:2 * h + 2])
                                nc.gpsimd.partition_broadcast(
                                    thrB[:, 12 + 2 * h:14 + 2 * h],
                                    thr12[0:1, 12 + 2 * h:14 + 2 * h])
                                tsel = pS.tile([16, 226], f32, tag="tsel",
                                               name="tsel", bufs=2)
                                nc.vector.scalar_tensor_tensor(
                                    tsel[:, 0:128], s16[:, h, 0:128],
                                    thrB[:, 2 * h:2 * h + 1],
                                    iota226[:, 0:128],
                                    op0=Alu.is_ge, op1=Alu.mult)
                                nc.vector.scalar_tensor_tensor(
                                    tsel[:, 128:226], s16[:, h, 128:226],
                                    thrB[:, 12 + 2 * h:13 + 2 * h],
                                    iota226[:, 128:226],
                                    op0=Alu.is_ge, op1=Alu.mult)
                                nc.vector.tensor_scalar_add(tsel[:], tsel[:],
                                                            -1.0)
                                nc.gpsimd.sparse_gather(
                                    selall[:, h, :], tsel[:],
                                    num_found=nfound[0:1, h:h + 1])
                    nc.vector.tensor_scalar(selall[:], selall[:], 0.0,
                                            float(NM - 1), Alu.max, Alu.min)
                    seli = pS.tile([16, HL, 32], i16)
                    nc.vector.tensor_copy(seli[:], selall[:])
                    for r in range(8):
                        nc.scalar.dma_start(idxs[16 * r:16 * (r + 1), :, :],
                                            seli[:])
                    for h in range(HL):
                        nc.gpsimd.dma_gather(
                            kvsel[:, h, :, :], memkv_d[h], idxs[:, h, :],
                            num_idxs=512, num_idxs_reg=512, elem_size=128)
                    if DBG:
                        nc.sync.dma_start(dbg_thr[:], thr12[:])
                        nc.sync.dma_start(dbg_idx[:], selall[:])
                        nc.sync.dma_start(dbg_sct[:], sct[:])
            # scS closed

            # ===== phase C: qkv GEMM =====
            QT = [512, 512, 512, 32]
            with tc.tile_pool(name="psC", bufs=3, space="PSUM") as psC:
                for fc in range(6):
                    dst = qT if fc < 3 else kTc
                    pair = fc % 3
                    for t, n0 in enumerate((0, 512, 1024, 1536)):
                        nn = QT[t]
                        g = psC.tile([128, 512], f32, space="PSUM",
                                     tag="gqk", name="gqk")
                        for cc in range(6):
                            nc.tensor.matmul(
                                g[:, 0:nn],
                                wqr[:, cc, 128 * fc:128 * (fc + 1)],
                                xTr[:, cc, n0:n0 + nn],
                                start=(cc == 0), stop=(cc == 5))
                        nc.vector.tensor_copy(dst[:, pair, n0:n0 + nn],
                                              g[:, 0:nn])
                nc.vector.memset(v_cur[:].bitcast(f32), 0.0)
                nc.vector.memset(v_sel[:].bitcast(f32), 0.0)
                for c in range(NCH):
                    rows = 128 if c < 12 else 32
                    nc.vector.memset(v_cur[0:rows, c, :, 64:65].bitcast(f32),
                                     1.0)
                for c in range(4):
                    rows = 128 if c < 3 else 127
                    nc.vector.memset(v_sel[0:rows, c, :, 64:65].bitcast(f32),
                                     1.0)
                for c in range(NCH):
                    rows = 128 if c < 12 else 32
                    gv = psC.tile([128, 384], f32, space="PSUM",
                                  tag="gv", name="gv")
                    for cc in range(6):
                        nc.tensor.matmul(
                            gv[0:rows, :],
                            xTr[:, cc, 128 * c:128 * c + rows],
                            wqr[:, cc, 768:1152],
                            start=(cc == 0), stop=(cc == 5))
                    nc.vector.tensor_copy(
                        v_cur[0:rows, c, :, 0:64],
                        gv[0:rows, :].rearrange("p (h e) -> p h e", h=HL))

            # ===== sel transposes + v_sel assembly =====
            with tc.tile_pool(name="psG", bufs=2, space="PSUM") as psG:
                for h in range(HL):
                    for c in range(4):
                        kps = psG.tile([64, 128], f32, space="PSUM",
                                       tag="kps", name="kps", bufs=2)
                        nc.tensor.transpose(kps[:], kvsel[:, h, c, 0:64],
                                            ident[:])
                        nc.vector.tensor_copy(
                            kTs[64 * (h % 2):64 * (h % 2) + 64, h // 2,
                                128 * c:128 * (c + 1)],
                            kps[:])
                    nc.vector.tensor_copy(v_sel[:, :, h, 0:64],
                                          kvsel[:, h, :, 64:128])
        # scA closed

        # ===== phase H: software-pipelined single-pass attention =====
        groups = []
        for ti, (n0, nn) in enumerate(AQT):
            for h in range(HL):
                for g in range(6):
                    groups.append((ti, h, g, n0, nn))
        NG = len(groups)

        with tc.tile_pool(name="scH", bufs=1) as scH:
            aT = scH.tile([128, 3, N], f32r)
            wpf = scH.tile([128, 6, C], f32)
            wpr = scH.tile([128, 6, C], f32r)
            bpf = scH.tile([1, C], f32)
            bpr = scH.tile([1, C], f32r)
            aTf = scH.tile([128, 3, NH], f32)
            aTr = scH.tile([128, 3, NH], f32r)
            sc_tiles = {}
            pbt_tiles = {}
            ot_tiles = {}

            def emit_score(i):
                ti, h, g, n0, nn = groups[i]
                hh, pr = 64 * (h % 2), h // 2
                if g == 0:
                    ot_tiles[(ti, h)] = psH.tile([65, 512], f32, space="PSUM",
                                                 tag="ot", name="ot", bufs=2)
                sc_ = psH.tile([128, 1536], f32, space="PSUM",
                               tag="sc", name="sc", bufs=2)
                sc_tiles[i] = sc_
                for gi, cidx in enumerate(range(3 * g, min(3 * g + 3, 17))):
                    mm = MC[cidx]
                    if cidx < 13:
                        lhs = kTc[hh:hh + 64, pr, 128 * cidx:128 * cidx + mm]
                    else:
                        sc0 = cidx - 13
                        lhs = kTs[hh:hh + 64, pr, 128 * sc0:128 * sc0 + mm]
                    nc.tensor.matmul(
                        sc_[0:mm, 512 * gi:512 * gi + nn],
                        lhs, qT[hh:hh + 64, pr, n0:n0 + nn],
                        start=True, stop=True)

            def emit_act(i):
                ti, h, g, n0, nn = groups[i]
                sc_ = sc_tiles.pop(i)
                pbt = pH.tile([128, 1536], f32r, tag="pbt", name="pbt", bufs=3)
                pbt_tiles[i] = pbt
                ng = len(range(3 * g, min(3 * g + 3, 17)))
                nc.scalar.activation(
                    pbt[:].rearrange("p (g f) -> p g f", g=3)[:, 0:ng, 0:nn],
                    sc_[:].rearrange("p (g f) -> p g f", g=3)[:, 0:ng, 0:nn],
                    Act.Exp, scale=0.125)

            def emit_av(i):
                ti, h, g, n0, nn = groups[i]
                pbt = pbt_tiles.pop(i)
                ot = ot_tiles[(ti, h)]
                for gi, cidx in enumerate(range(3 * g, min(3 * g + 3, 17))):
                    mm = MC[cidx]
                    if cidx < 13:
                        vl = v_cur[0:mm, cidx, h, :]
                    else:
                        vl = v_sel[0:mm, cidx - 13, h, :]
                    nc.tensor.matmul(
                        ot[:, 0:nn], vl, pbt[0:mm, 512 * gi:512 * gi + nn],
                        start=(cidx == 0), stop=(cidx == 16))
                if g < 5:
                    return
                hh, pr = 64 * (h % 2), h // 2
                rcp = pH.tile([1, 512], f32r, tag="rcp", name="rcp", bufs=2)
                with nc.allow_low_precision(reason="f32r recip for PE"):
                    nc.vector.reciprocal(rcp[0:1, 0:nn], ot[64:65, 0:nn])
                rsb = pH.tile([64, 512], f32r, tag="rsb", name="rsb", bufs=2)
                nc.gpsimd.partition_broadcast(rsb[0:64, 0:nn],
                                              rcp[0:1, 0:nn])
                nc.vector.tensor_tensor(aT[hh:hh + 64, pr, n0:n0 + nn],
                                        ot[0:64, 0:nn], rsb[0:64, 0:nn],
                                        op=Alu.mult)
                ot_tiles.pop((ti, h))
                if ti == 1 and h == HL - 1:
                    # other-half aT complete: masked sends + hidden RS
                    for s in range(2):
                        aTm = pH.tile([128, 3, NH], f32, tag="aTm",
                                      name="aTm", bufs=1)
                        nc.vector.tensor_scalar_mul(
                            aTm[:], aT[:, :, NH:N].bitcast(f32),
                            msk[:, s:s + 1])
                        nc.sync.dma_start(a_send[s], aTm[:])
                    nc.gpsimd.collective_compute(
                        "ReduceScatter", Alu.add,
                        ins=[a_send[:].opt()],
                        outs=[a_recv[:].opt()],
                        replica_groups=[[0, 1], [2, 3], [4, 5], [6, 7]],
                    )
                    # prefetch projection operands during attention
                    nc.sync.dma_start(
                        wpf[:],
                        wprojT_d[:].rearrange("(cc p) f -> p cc f", p=128))
                    nc.scalar.dma_start(bpf[:], bproj_d[:])
                    for cc in range(6):
                        nc.vector.tensor_copy(wpr[:, cc, :], wpf[:, cc, :])
                    nc.vector.tensor_copy(bpr[:], bpf[:])

            with tc.tile_pool(name="pH", bufs=1) as pH, \
                 tc.tile_pool(name="psH", bufs=1, space="PSUM") as psH:
                for i in range(NG + 2):
                    if i < NG:
                        emit_score(i)
                    if 1 <= i <= NG:
                        emit_act(i - 1)
                    if i >= 2:
                        emit_av(i - 2)

            nc.scalar.dma_start(aTf[:], a_recv[:])
            nc.vector.tensor_copy(aTr[:], aTf[:])

            if DBG:
                nc.sync.dma_start(dbg_aT[:], aT[:].bitcast(f32))

            # ===== phase P: projection =====
            with tc.tile_pool(name="pP", bufs=1) as pP, \
                 tc.tile_pool(name="psP", bufs=2, space="PSUM") as psP:
                for t in range(7):
                    rows = 128 if t < 6 else 16
                    yps = psP.tile([128, C], f32, space="PSUM", tag="yps",
                                   name="yps", bufs=2)
                    for c0, c1 in ((0, 512), (512, 768)):
                        nc.tensor.matmul(yps[0:rows, c0:c1],
                                         ones1[0:1, 0:rows],
                                         bpr[:, c0:c1], start=True, stop=False)
                        for cc in range(3):
                            nc.tensor.matmul(
                                yps[0:rows, c0:c1],
                                aT[:, cc, 128 * t:128 * t + rows],
                                wpr[:, cc, c0:c1],
                                start=False, stop=False)
                        for cc in range(3):
                            nc.tensor.matmul(
                                yps[0:rows, c0:c1],
                                aTr[:, cc, 128 * t:128 * t + rows],
                                wpr[:, 3 + cc, c0:c1],
                                start=False, stop=(cc == 2))
                    ysb = pP.tile([128, C], f32, tag="ysb", name="ysb", bufs=2)
                    nc.vector.tensor_copy(ysb[0:rows, :], yps[0:rows, :])
                    nc.sync.dma_start(out_d[128 * t:128 * t + rows, :],
                                      ysb[0:rows, :])

    nc.finalize()
    return nc


def _consts():
    ident = np.eye(128, dtype=np.float32)
    iota = np.zeros((16, 226), np.float32)
    for p in range(16):
        for f in range(128):
            iota[p, f] = p * 128 + f + 1
        for f in range(98):
            iota[p, 128 + f] = NB + p * 98 + f + 1
    return {"ident": ident, "iota226": iota}


def _get_nc():
    if "nc" not in _cache:
        _cache["nc"] = _build()
    return _cache["nc"]


def make_in_maps(x, bank_k, bank_v, prev_k, prev_v, w_qkv, w_proj, b_proj):
    x = np.asarray(x, np.float32)
    bank_k = np.asarray(bank_k, np.float32)
    bank_v = np.asarray(bank_v, np.float32)
    prev_k = np.asarray(prev_k, np.float32)
    prev_v = np.asarray(prev_v, np.float32)
    w_qkv = np.asarray(w_qkv, np.float32)
    w_proj = np.asarray(w_proj, np.float32)
    b_proj = np.asarray(b_proj, np.float32)
    consts = _consts()
    wprojT_full = np.ascontiguousarray(w_proj.T)
    in_maps = []
    for c in range(8):
        b, hg = c // 2, c % 2
        rows = np.concatenate([
            w_qkv[hg * 384:(hg + 1) * 384],
            w_qkv[C + hg * 384:C + (hg + 1) * 384],
            w_qkv[2 * C + hg * 384:2 * C + (hg + 1) * 384]], axis=0)
        own, oth = hg * NH, (1 - hg) * NH
        x_local = np.concatenate([x[b, own:own + NH], x[b, oth:oth + NH]],
                                 axis=0)
        wp_local = np.concatenate([
            wprojT_full[hg * 384:(hg + 1) * 384],
            wprojT_full[(1 - hg) * 384:(2 - hg) * 384]], axis=0)
        memk = np.concatenate([bank_k[b, 6 * hg:6 * hg + 6],
                               prev_k[b, 6 * hg:6 * hg + 6]], axis=1)
        memv = np.concatenate([bank_v[b, 6 * hg:6 * hg + 6],
                               prev_v[b, 6 * hg:6 * hg + 6]], axis=1)
        memkv = np.concatenate([memk, memv], axis=2)   # [6, 3616, 128]
        mskv = np.zeros((128, 2), np.float32)
        mskv[:, 1 - hg] = 1.0
        m = {
            "x": np.ascontiguousarray(x_local),
            "x0": np.ascontiguousarray(x[b, 0:1, :]),
            "wqkvT": np.ascontiguousarray(rows.T),
            "wprojT": np.ascontiguousarray(wp_local),
            "bproj": b_proj.reshape(1, C),
            "memkv": np.ascontiguousarray(memkv),
            "msk": mskv,
        }
        m.update(consts)
        in_maps.append(m)
    return in_maps


def kernel(x, bank_k, bank_v, prev_k, prev_v, w_qkv, w_proj, b_proj,
           _trace=False):
    from concourse.bass_utils import run_bass_kernel_spmd
    nc = _get_nc()
    in_maps = make_in_maps(x, bank_k, bank_v, prev_k, prev_v,
                           w_qkv, w_proj, b_proj)
    res = run_bass_kernel_spmd(nc, in_maps, core_ids=list(range(8)),
                               trace=_trace)
    out = np.zeros((B, N, C), np.float32)
    for c in range(8):
        b, hg = c // 2, c % 2
        out[b, hg * NH:(hg + 1) * NH, :] = res.results[c]["out"]
    if _trace:
        return out, res
    return out
